# revision 1
# baseline (speedup 1.0000x reference)
"""MixHop GNN (2 layers, 2 adjacencies, hops 0..2) on 8 trn2 NeuronCores.

Sharding: nodes row-partitioned across 8 cores (6250 rows each). Each SpMM
is computed for the core's destination rows only; gather sources are
full-size tables (Y12 computed redundantly per core; later tables
assembled via AllGather). The SpMM maps to TensorE as a segment matmul:
for each 128-edge tile, gather source rows (indirect DMA), build a
val-scaled one-hot matrix M[e, r] = val[e] * (lrow[e] == r) on the vector
engine, and accumulate M^T @ G into a PSUM block of 128 destination rows.

Self-contained: only numpy + concourse (environment packages).
"""
import numpy as np

import concourse.bass as bass
from concourse import mybir
from concourse.bass import IndirectOffsetOnAxis
from concourse.bass_utils import run_bass_kernel_spmd
from concourse.tile import TileContext

F32 = mybir.dt.float32
BF16 = mybir.dt.bfloat16
I32 = mybir.dt.int32
AL = mybir.AluOpType

N = 50000
E = 800000
NCORES = 8
R = N // NCORES          # 6250 rows per core
BLK = 128
NB = (R + BLK - 1) // BLK  # 49 blocks (48 full + 106)
LAM = 0.5
EPS = 1e-5
G = 6                    # gathered 128-row tiles per indirect DMA call
P = 128

TRACE = False            # set by test harness for profiling runs
LAST_RESULT = {}


# ---------------------------------------------------------------- BIR post-pass
from concourse import mybir

ASYNC_OPCODES = {"DMACopy", "CollectiveCompute", "DMAGatherAnt",
                 "DMAScatterAddAnt", "DMATransposeAnt"}


def _cap(inst) -> int:
    if inst.opcode in ("EventSemaphore", "NoOp"):
        return 999
    return 1


def fix_waits(nc, verbose=False):
    # --- collect streams (blocks concatenated in listed order; Tile output
    # is straight-line per engine)
    all_bbs = [bb for fn in nc.m.functions for bb in fn.blocks]
    streams = {}
    order = {}
    for bb in all_bbs:
        for inst in bb.instructions:
            streams.setdefault(inst.engine, []).append(inst)

    unsafe = set()
    wait_list = {}
    upd_list = {}
    for eng, insts in streams.items():
        for inst in insts:
            si = inst.sync_info
            ws, us = [], []
            if si:
                for w in (si.on_wait or []):
                    if getattr(w, "wait_mode", None) == "sem-ge-imm" and isinstance(
                            getattr(w, "wait_value", None), int):
                        ws.append((w.id, w.wait_value, w))
                    else:
                        ws.append((w.id, None, w))
                        unsafe.add(w.id)
                for u in (si.on_update or []):
                    um = getattr(u, "update_mode", None)
                    uv = getattr(u, "update_value", None)
                    if um == "sem-add-imm" and isinstance(uv, int):
                        us.append((u.id, uv))
                    elif um == "sem-inc":
                        us.append((u.id, 1))
                    else:
                        us.append((u.id, 0))
                        unsafe.add(u.id)
            wait_list[id(inst)] = ws
            upd_list[id(inst)] = us

    engines = list(streams.keys())
    ptr = {e: 0 for e in engines}
    vc = {e: {} for e in engines}
    sem_level = {}
    # per sem: (cum_after list, prefix-max snapshot list)
    sem_cums = {}
    sem_snaps = {}

    def knowledge(s, v):
        cums = sem_cums.get(s)
        if not cums:
            return None
        # smallest index with cum >= v
        import bisect
        i = bisect.bisect_left(cums, v)
        if i >= len(cums):
            i = len(cums) - 1
        return sem_snaps[s][i]

    n_dropped = 0
    progressed = True
    while progressed:
        progressed = False
        for eng in engines:
            insts = streams[eng]
            while ptr[eng] < len(insts):
                inst = insts[ptr[eng]]
                ws = wait_list[id(inst)]
                # resolvable?
                ok = True
                for (s, v, w) in ws:
                    if s in unsafe or v is None:
                        continue
                    if sem_level.get(s, 0) < v:
                        ok = False
                        break
                if not ok:
                    break
                myvc = vc[eng]
                kept = []
                # engine sems first: their knowledge usually implies the
                # DMA-lane waits, letting us drop the latter
                ws = sorted(ws, key=lambda t: str(
                    getattr(t[2], "ant_name", "")).startswith("DMA"))
                for (s, v, w) in ws:
                    if s not in unsafe and v is not None and myvc.get(s, 0) >= v:
                        n_dropped += 1
                        continue
                    kept.append(w)
                    if s in unsafe or v is None:
                        continue
                    k = knowledge(s, v)
                    if k:
                        for ks, kv in k.items():
                            if myvc.get(ks, 0) < kv:
                                myvc[ks] = kv
                    if myvc.get(s, 0) < v:
                        myvc[s] = v
                si = inst.sync_info
                if si and len(kept) != len(si.on_wait or []):
                    inst.sync_info = mybir.SyncInfo(
                        on_wait=kept, on_update=list(si.on_update or []))
                # register updates
                us = upd_list[id(inst)]
                if us:
                    is_async = inst.opcode in ASYNC_OPCODES
                    for (s, u) in us:
                        lvl = sem_level.get(s, 0) + u
                        sem_level[s] = lvl
                        if s not in unsafe:
                            snap = dict(myvc)
                            snap[s] = lvl
                            cums = sem_cums.setdefault(s, [])
                            snaps = sem_snaps.setdefault(s, [])
                            if snaps:
                                prev = snaps[-1]
                                for ks, kv in prev.items():
                                    if snap.get(ks, 0) < kv:
                                        snap[ks] = kv
                            cums.append(lvl)
                            snaps.append(snap)
                            if not is_async:
                                myvc[s] = lvl
                ptr[eng] += 1
                progressed = True

    stuck = sum(len(streams[e]) - ptr[e] for e in engines)
    # --- cap remaining waits with carriers
    uid = 0
    n_carriers = 0
    for bb in all_bbs:
        new_insts = []
        for inst in bb.instructions:
            si = inst.sync_info
            waits = list(si.on_wait) if (si and si.on_wait) else []
            cap = _cap(inst)
            if len(waits) > cap:
                keep = waits[len(waits) - cap:]
                for w in waits[: len(waits) - cap]:
                    uid += 1
                    new_insts.append(mybir.InstEventSemaphore(
                        name=f"waitfix-{uid}",
                        engine=inst.engine, ins=[], outs=[],
                        sync_info=mybir.SyncInfo(on_wait=[w], on_update=[]),
                    ))
                    n_carriers += 1
                inst.sync_info = mybir.SyncInfo(
                    on_wait=keep, on_update=list(si.on_update or []))
            new_insts.append(inst)
        bb.instructions = new_insts
    if verbose:
        print(f"fix_waits: dropped {n_dropped} redundant waits, "
              f"{n_carriers} carriers, {stuck} unprocessed")
    return nc


# ---------------------------------------------------------------- preprocessing
def _prep_adj(row, col, val):
    """Partition edges by destination core, group by 128-row block, sort by
    col within block, pad each (core, block) to a common per-block tile
    count. Returns per-core [128, T] arrays (col/lrow/val) + block schedule."""
    core = row // R
    rloc = row - core * R
    blk = rloc // BLK
    lrow = rloc - blk * BLK

    counts = np.zeros((NCORES, NB), np.int64)
    np.add.at(counts, (core, blk), 1)
    tb = np.maximum(1, (counts.max(axis=0) + BLK - 1) // BLK)  # [NB]
    tstart = np.concatenate([[0], np.cumsum(tb)])
    T = int(tstart[-1])

    cols = np.zeros((NCORES, T * BLK), np.int32)
    lrows = np.zeros((NCORES, T * BLK), np.float32)
    vals = np.zeros((NCORES, T * BLK), np.float32)

    order = np.lexsort((col, blk, core))
    core_s, blk_s = core[order], blk[order]
    col_s, lrow_s, val_s = col[order], lrow[order], val[order]
    # boundaries per (core, block)
    key = core_s * NB + blk_s
    starts = np.searchsorted(key, np.arange(NCORES * NB))
    ends = np.searchsorted(key, np.arange(NCORES * NB) + 1)
    for c in range(NCORES):
        for b in range(NB):
            s, e = starts[c * NB + b], ends[c * NB + b]
            n = e - s
            off = int(tstart[b]) * BLK
            cols[c, off:off + n] = col_s[s:e]
            lrows[c, off:off + n] = lrow_s[s:e]
            vals[c, off:off + n] = val_s[s:e]
    # [T*128] -> [128, T] (edge e of tile t sits at [e, t])
    cols = cols.reshape(NCORES, T, BLK).transpose(0, 2, 1).copy()
    lrows = lrows.reshape(NCORES, T, BLK).transpose(0, 2, 1).copy()
    vals = vals.reshape(NCORES, T, BLK).transpose(0, 2, 1).copy()
    sched = [(int(tstart[b]), int(tb[b])) for b in range(NB)]
    return cols, lrows, vals, sched, T


# ---------------------------------------------------------------- device program
def _build(TA, TN, sched_a, sched_n):
    nc = bass.Bass(num_devices=NCORES)
    groups = [list(range(NCORES))]

    # ---- external I/O
    xt = nc.declare_dram_parameter("xt", [P, N], F32, isOutput=False)
    xl_in = nc.declare_dram_parameter("xl", [P, R], F32, isOutput=False)
    a_col = nc.declare_dram_parameter("a_col", [P, TA], I32, isOutput=False)
    a_lrow = nc.declare_dram_parameter("a_lrow", [P, TA], F32, isOutput=False)
    a_val = nc.declare_dram_parameter("a_val", [P, TA], F32, isOutput=False)
    n_col = nc.declare_dram_parameter("n_col", [P, TN], I32, isOutput=False)
    n_lrow = nc.declare_dram_parameter("n_lrow", [P, TN], F32, isOutput=False)
    n_val = nc.declare_dram_parameter("n_val", [P, TN], F32, isOutput=False)
    iota_in = nc.declare_dram_parameter("iota", [P, P], F32, isOutput=False)
    ident_in = nc.declare_dram_parameter("ident", [P, P], F32, isOutput=False)
    ones1_in = nc.declare_dram_parameter("ones1", [1, P], F32, isOutput=False)
    onesb_in = nc.declare_dram_parameter("onesb", [P, 1], BF16, isOutput=False)
    w1h_in = nc.declare_dram_parameter("w1h", [128, 256], F32, isOutput=False)
    w10_in = nc.declare_dram_parameter("w10", [128, 128], F32, isOutput=False)
    b1h_in = nc.declare_dram_parameter("b1h", [1, 256], F32, isOutput=False)
    b10_in = nc.declare_dram_parameter("b10", [1, 128], F32, isOutput=False)
    w2h_in = nc.declare_dram_parameter("w2h", [384, 256], F32, isOutput=False)
    w20_in = nc.declare_dram_parameter("w20", [384, 128], F32, isOutput=False)
    b2h_in = nc.declare_dram_parameter("b2h", [1, 256], F32, isOutput=False)
    b20_in = nc.declare_dram_parameter("b20", [1, 128], F32, isOutput=False)
    bng_in = nc.declare_dram_parameter("bng", [1, 384], F32, isOutput=False)
    bnb_in = nc.declare_dram_parameter("bnb", [1, 384], F32, isOutput=False)
    fpw_in = nc.declare_dram_parameter("fpw", [384, 128], F32, isOutput=False)
    fpb_in = nc.declare_dram_parameter("fpb", [1, 128], F32, isOutput=False)
    out_ext = nc.declare_dram_parameter("out", [R, 128], F32, isOutput=True)

    with TileContext(nc) as tc:
        with (
            tc.tile_pool(name="consts", bufs=1) as consts,
            tc.tile_pool(name="xtc", bufs=3) as xtc,
            tc.tile_pool(name="gp", bufs=8) as gp,
            tc.tile_pool(name="mp", bufs=8) as mp,
            tc.tile_pool(name="wk", bufs=3) as wk,
            tc.tile_pool(name="ps", bufs=3, space="PSUM") as ps,
            tc.tile_pool(name="pm", bufs=2, space="PSUM") as pm,
            tc.tile_pool(name="dram", bufs=1, space="DRAM") as dram,
        ):
            # ---------------- DRAM scratch
            y12 = dram.tile([N, 256], BF16)
            ta_loc = dram.tile([R, 128], BF16)
            tn_loc = dram.tile([R, 128], BF16)
            z12_loc = dram.tile([R, 256], BF16)
            tpa_loc = dram.tile([R, 128], BF16)
            tpn_loc = dram.tile([R, 128], BF16)
            bn_loc = dram.tile([1, 768], F32)
            ta_full = dram.tile([N, 128], BF16, addr_space="Shared")
            tn_full = dram.tile([N, 128], BF16, addr_space="Shared")
            z12_full = dram.tile([N, 256], BF16, addr_space="Shared")
            tpa_full = dram.tile([N, 128], BF16, addr_space="Shared")
            tpn_full = dram.tile([N, 128], BF16, addr_space="Shared")
            bn_full = dram.tile([1, 768], F32, addr_space="Shared")

            # ---------------- constants to SBUF
            def cload(src, shape, dtype=F32):
                t = consts.tile(shape, dtype, name=f"c_{src.name}")
                nc.sync.dma_start(out=t[:], in_=src[:])
                return t

            iota = cload(iota_in, [P, P])
            ident = cload(ident_in, [P, P])
            ones1 = cload(ones1_in, [1, P])
            onesb = cload(onesb_in, [P, 1], BF16)
            w1h = cload(w1h_in, [128, 256])
            w10 = cload(w10_in, [128, 128])
            b1h = cload(b1h_in, [1, 256])
            b10 = cload(b10_in, [1, 128])
            b2h = cload(b2h_in, [1, 256])
            b20 = cload(b20_in, [1, 128])
            bng = cload(bng_in, [1, 384])
            bnb = cload(bnb_in, [1, 384])
            fpb = cload(fpb_in, [1, 128])
            w2h_sb, w20_sb, fpw_sb = [], [], []
            for k in range(3):
                t = consts.tile([128, 256], F32, name=f"w2h{k}")
                nc.sync.dma_start(out=t[:], in_=w2h_in[k * 128:(k + 1) * 128, :])
                w2h_sb.append(t)
                t = consts.tile([128, 128], F32, name=f"w20{k}")
                nc.sync.dma_start(out=t[:], in_=w20_in[k * 128:(k + 1) * 128, :])
                w20_sb.append(t)
                t = consts.tile([128, 128], F32, name=f"fpw{k}")
                nc.sync.dma_start(out=t[:], in_=fpw_in[k * 128:(k + 1) * 128, :])
                fpw_sb.append(t)

            def eload(src, T, dtype):
                t = consts.tile([P, T], dtype, name=f"e_{src.name}")
                nc.sync.dma_start(out=t[:], in_=src[:])
                return t

            acol = eload(a_col, TA, I32)
            alrow = eload(a_lrow, TA, F32)
            aval = eload(a_val, TA, F32)
            ncol = eload(n_col, TN, I32)
            nlrow = eload(n_lrow, TN, F32)
            nval = eload(n_val, TN, F32)

            # persistent per-node-block SBUF arrays
            h1_all = consts.tile([P, NB * 384], BF16, name="h1_all")
            h2a = consts.tile([P, NB * 128], BF16, name="h2a")   # 1.5*Z0
            h2b = consts.tile([P, NB * 128], BF16, name="h2b")   # S'_1 combined
            stats = consts.tile([1, 768], F32, name="stats")
            nc.vector.memset(stats[:], 0.0)
            scaleB = consts.tile([P, 384], F32, name="scaleB")
            shiftB = consts.tile([P, 384], F32, name="shiftB")

            def nb_of(b):
                return BLK if b < NB - 1 else R - BLK * (NB - 1)

            # broadcast biases to all partitions once (replaces per-tile
            # bias matmuls); hop0 biases pre-scaled by (1+LAM)
            s10 = consts.tile([1, 128], F32, name="s10")
            nc.vector.tensor_scalar(out=s10[:], in0=b10[:], scalar1=1.0 + LAM,
                                    scalar2=None, op0=AL.mult)
            s20 = consts.tile([1, 128], F32, name="s20")
            nc.vector.tensor_scalar(out=s20[:], in0=b20[:], scalar1=1.0 + LAM,
                                    scalar2=None, op0=AL.mult)
            bcast = {}
            for nm, bsrc, wdt in (("b1h", b1h, 256), ("b2h", b2h, 256),
                                  ("b10s", s10, 128), ("b20s", s20, 128),
                                  ("fpb", fpb, 128)):
                pbx = pm.tile([P, 256], F32, tag="pmm", name=f"pb_{nm}")
                nc.tensor.matmul(out=pbx[:, :wdt], lhsT=ones1[:], rhs=bsrc[:],
                                 start=True, stop=True)
                bt = consts.tile([P, wdt], F32, name=f"bb_{nm}")
                nc.vector.tensor_copy(out=bt[:], in_=pbx[:, :wdt])
                bcast[nm] = bt

            # ---------------- stage 1: Y12 = X @ [W1;W2]^T + b (full, redundant)
            NT = (N + P - 1) // P     # 391 node tiles
            CH = 8                    # tiles per XT chunk load
            for c0 in range(0, NT, CH):
                c1 = min(c0 + CH, NT)
                w = min(N - c0 * P, (c1 - c0) * P)
                xchunk = xtc.tile([P, CH * P], F32, tag="xchunk")
                nc.sync.dma_start(out=xchunk[:, :w], in_=xt[:, c0 * P:c0 * P + w])
                for t in range(c0, c1):
                    tn = min(P, N - t * P)
                    off = (t - c0) * P
                    pz = pm.tile([P, 256], F32, tag="pmm")
                    nc.tensor.matmul(out=pz[:tn, :], lhsT=xchunk[:, off:off + tn],
                                     rhs=w1h[:], start=True, stop=True)
                    yt = wk.tile([P, 256], BF16, tag="yt")
                    nc.vector.tensor_tensor(out=yt[:tn, :], in0=pz[:tn, :],
                                            in1=bcast["b1h"][:tn, :], op=AL.add)
                    nc.sync.dma_start(out=y12[t * P:t * P + tn, :], in_=yt[:tn, :])

            # ---------------- spmm helper
            def spmm_block(b, sched, colt, lrowt, valt, table, elem, psum):
                # NOTE: hardware indirect DMA consumes ONE index per
                # partition per call (sim's multi-index semantics do not
                # hold) — gather one 128-edge tile per call.
                t0, tb = sched[b]
                for i in range(tb):
                    tt = t0 + i
                    gt = gp.tile([P, 256], BF16, tag="gt")
                    nc.gpsimd.indirect_dma_start(
                        out=gt[:, :elem], out_offset=None,
                        in_=table[:],
                        in_offset=IndirectOffsetOnAxis(
                            ap=colt[:, tt:tt + 1], axis=0),
                    )
                    m = mp.tile([P, P], BF16, tag="m")
                    nc.vector.tensor_scalar(
                        out=m[:], in0=iota[:],
                        scalar1=lrowt[:, tt:tt + 1], scalar2=valt[:, tt:tt + 1],
                        op0=AL.is_equal, op1=AL.mult)
                    nc.tensor.matmul(out=psum[:, :elem], lhsT=m[:],
                                     rhs=gt[:, :elem],
                                     start=(i == 0), stop=(i == tb - 1))

            # ---------------- stage 2a: adjacency A phase 1 (hop1 + T_A)
            for b in range(NB):
                nb = nb_of(b)
                pa = ps.tile([P, 256], F32, tag="sp")
                spmm_block(b, sched_a, acol, alrow, aval, y12, 256, pa)
                # S_A1 -> h1 hop1 slice (store; combined later with ND)
                nc.vector.tensor_copy(
                    out=h1_all[:, b * 384 + 128:b * 384 + 256], in_=pa[:, 0:128])
                tsb = wk.tile([P, 128], BF16, tag="tsb")
                nc.vector.tensor_copy(out=tsb[:], in_=pa[:, 128:256])
                nc.sync.dma_start(out=ta_loc[b * BLK:b * BLK + nb, :], in_=tsb[:nb, :])
            nc.gpsimd.collective_compute(
                "AllGather", AL.bypass, replica_groups=groups,
                ins=[ta_loc[:]], outs=[ta_full[:]])

            # ---------------- stage 2b: adjacency ND phase 1
            for b in range(NB):
                nb = nb_of(b)
                pn = ps.tile([P, 256], F32, tag="sp")
                spmm_block(b, sched_n, ncol, nlrow, nval, y12, 256, pn)
                # h1 hop1 = S_A1 + 0.5*S_N1
                sl = h1_all[:, b * 384 + 128:b * 384 + 256]
                nc.vector.scalar_tensor_tensor(
                    out=sl, in0=pn[:, 0:128], scalar=LAM, in1=sl,
                    op0=AL.mult, op1=AL.add)
                tsb = wk.tile([P, 128], BF16, tag="tsb")
                nc.vector.tensor_copy(out=tsb[:], in_=pn[:, 128:256])
                nc.sync.dma_start(out=tn_loc[b * BLK:b * BLK + nb, :], in_=tsb[:nb, :])
            nc.gpsimd.collective_compute(
                "AllGather", AL.bypass, replica_groups=groups,
                ins=[tn_loc[:]], outs=[tn_full[:]])

            # ---------------- stage 4: layer-1 hop0 (local rows)
            NLC = (R + CH * P - 1) // (CH * P)
            for c0 in range(NLC):
                s = c0 * CH * P
                w = min(CH * P, R - s)
                xl = xtc.tile([P, CH * P], F32, tag="xchunk")
                # xl_in: per-core local columns of XT (core-dependent data,
                # same shape on every core)
                nc.sync.dma_start(out=xl[:, :w], in_=xl_in[:, s:s + w])
                for b in range(c0 * CH, min((c0 + 1) * CH, NB)):
                    nb = nb_of(b)
                    off = b * P - s
                    p0 = pm.tile([P, 128], F32, tag="pmm")
                    nc.tensor.matmul(out=p0[:nb, :], lhsT=xl[:, off:off + nb],
                                     rhs=w10[:], start=True, stop=True)
                    nc.vector.scalar_tensor_tensor(
                        out=h1_all[:nb, b * 384:b * 384 + 128], in0=p0[:nb, :],
                        scalar=1.0 + LAM, in1=bcast["b10s"][:nb, :],
                        op0=AL.mult, op1=AL.add)

            # ---------------- stage 5: phase 2 (hop2) + stats
            for b in range(NB):
                nb = nb_of(b)
                pa = ps.tile([P, 256], F32, tag="sp")
                spmm_block(b, sched_a, acol, alrow, aval, ta_full, 128, pa)
                nc.vector.tensor_copy(
                    out=h1_all[:, b * 384 + 256:b * 384 + 384], in_=pa[:, 0:128])
            for b in range(NB):
                nb = nb_of(b)
                pn = ps.tile([P, 256], F32, tag="sp")
                spmm_block(b, sched_n, ncol, nlrow, nval, tn_full, 128, pn)
                sl = h1_all[:, b * 384 + 256:b * 384 + 384]
                nc.vector.scalar_tensor_tensor(
                    out=sl, in0=pn[:, 0:128], scalar=LAM, in1=sl,
                    op0=AL.mult, op1=AL.add)
                # stats for this fully-assembled block
                hsl = h1_all[:, b * 384:b * 384 + 384]
                sq = wk.tile([P, 384], BF16, tag="sq")
                nc.vector.tensor_tensor(out=sq[:nb, :], in0=hsl[:nb],
                                        in1=hsl[:nb], op=AL.mult)
                pst = pm.tile([1, 384], F32, tag="pst", bufs=1)
                psq = pm.tile([1, 384], F32, tag="psq", bufs=1)
                nc.tensor.matmul(out=pst[:], lhsT=onesb[:nb, :], rhs=hsl[:nb],
                                 start=True, stop=True)
                nc.tensor.matmul(out=psq[:], lhsT=onesb[:nb, :], rhs=sq[:nb, :],
                                 start=True, stop=True)
                nc.vector.tensor_tensor(out=stats[:, 0:384], in0=stats[:, 0:384],
                                        in1=pst[:], op=AL.add)
                nc.vector.tensor_tensor(out=stats[:, 384:768], in0=stats[:, 384:768],
                                        in1=psq[:], op=AL.add)

            # ---------------- stage 6: BN finalize (allreduce + scale/shift)
            nc.sync.dma_start(out=bn_loc[:], in_=stats[:])
            nc.gpsimd.collective_compute(
                "AllReduce", AL.add, replica_groups=groups,
                ins=[bn_loc[:]], outs=[bn_full[:]])
            bnr = wk.tile([1, 768], F32, tag="bnr", bufs=1)
            nc.sync.dma_start(out=bnr[:], in_=bn_full[:])
            mean = wk.tile([1, 384], F32, tag="bn1", bufs=1)
            var = wk.tile([1, 384], F32, tag="bn2", bufs=1)
            scl = wk.tile([1, 384], F32, tag="bn3", bufs=1)
            shf = wk.tile([1, 384], F32, tag="bn4", bufs=1)
            nc.vector.tensor_scalar(out=mean[:], in0=bnr[:, 0:384],
                                    scalar1=1.0 / N, scalar2=None, op0=AL.mult)
            nc.vector.tensor_scalar(out=var[:], in0=bnr[:, 384:768],
                                    scalar1=1.0 / N, scalar2=None, op0=AL.mult)
            # var = E[x^2] - mean^2 ; scl = gamma/sqrt(var+eps)
            nc.vector.tensor_tensor(out=scl[:], in0=mean[:], in1=mean[:], op=AL.mult)
            nc.vector.tensor_tensor(out=var[:], in0=var[:], in1=scl[:], op=AL.subtract)
            nc.vector.tensor_scalar(out=var[:], in0=var[:], scalar1=EPS,
                                    scalar2=None, op0=AL.add)
            nc.scalar.sqrt(out=var[:], in_=var[:])
            nc.vector.reciprocal(out=var[:], in_=var[:])
            nc.vector.tensor_tensor(out=scl[:], in0=bng[:], in1=var[:], op=AL.mult)
            nc.vector.tensor_tensor(out=shf[:], in0=mean[:], in1=scl[:], op=AL.mult)
            nc.vector.tensor_tensor(out=shf[:], in0=bnb[:], in1=shf[:], op=AL.subtract)
            pb1 = pm.tile([P, 384], F32, tag="pmm")
            nc.tensor.matmul(out=pb1[:], lhsT=ones1[:], rhs=scl[:], start=True, stop=True)
            nc.vector.tensor_copy(out=scaleB[:], in_=pb1[:])
            pb2 = pm.tile([P, 384], F32, tag="pmm")
            nc.tensor.matmul(out=pb2[:], lhsT=ones1[:], rhs=shf[:], start=True, stop=True)
            nc.vector.tensor_copy(out=shiftB[:], in_=pb2[:])

            # ---------------- stage 7: BN apply + relu + layer-2 linears
            for b in range(NB):
                nb = nb_of(b)
                hsl = h1_all[:, b * 384:b * 384 + 384]
                hb = wk.tile([P, 384], F32, tag="hb")
                nc.vector.tensor_tensor(out=hb[:nb, :], in0=hsl[:nb],
                                        in1=scaleB[:nb, :], op=AL.mult)
                nc.vector.tensor_tensor(out=hb[:nb, :], in0=hb[:nb, :],
                                        in1=shiftB[:nb, :], op=AL.add)
                nc.vector.tensor_scalar(out=hb[:nb, :], in0=hb[:nb, :],
                                        scalar1=0.0, scalar2=None, op0=AL.max)
                hbT = wk.tile([P, 384], F32, tag="hbT")
                for k in range(3):
                    pt = pm.tile([P, 128], F32, tag="ptr", bufs=1)
                    nc.tensor.transpose(out=pt[:, :nb],
                                        in_=hb[:nb, k * 128:(k + 1) * 128],
                                        identity=ident[:nb, :nb])
                    nc.vector.tensor_copy(out=hbT[:, k * 128:k * 128 + nb],
                                          in_=pt[:, :nb])
                # Z12 = hb @ [W21;W22]^T + b
                pz = pm.tile([P, 256], F32, tag="pmm")
                for k in range(3):
                    nc.tensor.matmul(out=pz[:nb, :],
                                     lhsT=hbT[:, k * 128:k * 128 + nb],
                                     rhs=w2h_sb[k][:], start=(k == 0), stop=(k == 2))
                zt = wk.tile([P, 256], BF16, tag="zt")
                nc.vector.tensor_tensor(out=zt[:nb, :], in0=pz[:nb, :],
                                        in1=bcast["b2h"][:nb, :], op=AL.add)
                nc.sync.dma_start(out=z12_loc[b * BLK:b * BLK + nb, :], in_=zt[:nb, :])
                # Z0 (hop0) scaled by 1.5
                p0 = pm.tile([P, 128], F32, tag="pmm")
                for k in range(3):
                    nc.tensor.matmul(out=p0[:nb, :],
                                     lhsT=hbT[:, k * 128:k * 128 + nb],
                                     rhs=w20_sb[k][:], start=(k == 0), stop=(k == 2))
                nc.vector.scalar_tensor_tensor(
                    out=h2a[:nb, b * 128:(b + 1) * 128], in0=p0[:nb, :],
                    scalar=1.0 + LAM, in1=bcast["b20s"][:nb, :],
                    op0=AL.mult, op1=AL.add)
            nc.gpsimd.collective_compute(
                "AllGather", AL.bypass, replica_groups=groups,
                ins=[z12_loc[:]], outs=[z12_full[:]])

            # ---------------- stage 9: layer-2 phase 1 (hop1' + T')
            for b in range(NB):
                nb = nb_of(b)
                pa = ps.tile([P, 256], F32, tag="sp")
                spmm_block(b, sched_a, acol, alrow, aval, z12_full, 256, pa)
                nc.vector.tensor_copy(out=h2b[:, b * 128:(b + 1) * 128],
                                      in_=pa[:, 0:128])
                tsb = wk.tile([P, 128], BF16, tag="tsb")
                nc.vector.tensor_copy(out=tsb[:], in_=pa[:, 128:256])
                nc.sync.dma_start(out=tpa_loc[b * BLK:b * BLK + nb, :], in_=tsb[:nb, :])
            nc.gpsimd.collective_compute(
                "AllGather", AL.bypass, replica_groups=groups,
                ins=[tpa_loc[:]], outs=[tpa_full[:]])
            for b in range(NB):
                nb = nb_of(b)
                pn = ps.tile([P, 256], F32, tag="sp")
                spmm_block(b, sched_n, ncol, nlrow, nval, z12_full, 256, pn)
                sl = h2b[:, b * 128:(b + 1) * 128]
                nc.vector.scalar_tensor_tensor(
                    out=sl, in0=pn[:, 0:128], scalar=LAM, in1=sl,
                    op0=AL.mult, op1=AL.add)
                tsb = wk.tile([P, 128], BF16, tag="tsb")
                nc.vector.tensor_copy(out=tsb[:], in_=pn[:, 128:256])
                nc.sync.dma_start(out=tpn_loc[b * BLK:b * BLK + nb, :], in_=tsb[:nb, :])
            nc.gpsimd.collective_compute(
                "AllGather", AL.bypass, replica_groups=groups,
                ins=[tpn_loc[:]], outs=[tpn_full[:]])

            # ---------------- stage 10: layer-2 phase 2 + final projection
            for b in range(NB):
                nb = nb_of(b)
                pa = ps.tile([P, 256], F32, tag="sp")
                spmm_block(b, sched_a, acol, alrow, aval, tpa_full, 128, pa)
                pn = ps.tile([P, 256], F32, tag="sp")
                spmm_block(b, sched_n, ncol, nlrow, nval, tpn_full, 128, pn)
                hb2 = wk.tile([P, 384], F32, tag="hb")
                nc.vector.tensor_copy(out=hb2[:nb, 0:128],
                                      in_=h2a[:nb, b * 128:(b + 1) * 128])
                nc.vector.tensor_copy(out=hb2[:nb, 128:256],
                                      in_=h2b[:nb, b * 128:(b + 1) * 128])
                nc.vector.tensor_copy(out=hb2[:nb, 256:384], in_=pa[:nb, 0:128])
                nc.vector.scalar_tensor_tensor(
                    out=hb2[:nb, 256:384], in0=pn[:nb, 0:128], scalar=LAM,
                    in1=hb2[:nb, 256:384], op0=AL.mult, op1=AL.add)
                hbT = wk.tile([P, 384], F32, tag="hbT")
                for k in range(3):
                    pt = pm.tile([P, 128], F32, tag="ptr", bufs=1)
                    nc.tensor.transpose(out=pt[:, :nb],
                                        in_=hb2[:nb, k * 128:(k + 1) * 128],
                                        identity=ident[:nb, :nb])
                    nc.vector.tensor_copy(out=hbT[:, k * 128:k * 128 + nb],
                                          in_=pt[:, :nb])
                po = pm.tile([P, 128], F32, tag="pmm")
                for k in range(3):
                    nc.tensor.matmul(out=po[:nb, :],
                                     lhsT=hbT[:, k * 128:k * 128 + nb],
                                     rhs=fpw_sb[k][:], start=(k == 0), stop=(k == 2))
                osb = wk.tile([P, 128], F32, tag="osb")
                nc.vector.tensor_tensor(out=osb[:nb, :], in0=po[:nb, :],
                                        in1=bcast["fpb"][:nb, :], op=AL.add)
                nc.sync.dma_start(out=out_ext[b * BLK:b * BLK + nb, :], in_=osb[:nb, :])

    return nc


def kernel(x, val, nd_val,
           l1_W0, l1_b0, l1_W1, l1_b1, l1_W2, l1_b2,
           l2_W0, l2_b0, l2_W1, l2_b1, l2_W2, l2_b2,
           bn_gamma, bn_beta, fp_W, fp_b,
           row, col, nd_row, nd_col):
    x = np.asarray(x, np.float32)
    row = np.asarray(row, np.int32); col = np.asarray(col, np.int32)
    val = np.asarray(val, np.float32)
    nd_row = np.asarray(nd_row, np.int32); nd_col = np.asarray(nd_col, np.int32)
    nd_val = np.asarray(nd_val, np.float32)

    a_cols, a_lrows, a_vals, sched_a, TA = _prep_adj(row, col, val)
    n_cols, n_lrows, n_vals, sched_n, TN = _prep_adj(nd_row, nd_col, nd_val)

    import ml_dtypes
    xt = np.ascontiguousarray(x.T)                       # [128, N]
    iota = np.tile(np.arange(P, dtype=np.float32)[None, :], (P, 1))
    ident = np.eye(P, dtype=np.float32)
    ones1 = np.ones((1, P), np.float32)
    onesb = np.ones((P, 1), ml_dtypes.bfloat16)
    w1h = np.ascontiguousarray(np.concatenate([l1_W1, l1_W2], 0).T, dtype=np.float32)
    b1h = np.concatenate([l1_b1, l1_b2])[None, :].astype(np.float32)
    w10 = np.ascontiguousarray(np.asarray(l1_W0).T, dtype=np.float32)
    b10 = np.asarray(l1_b0)[None, :].astype(np.float32)
    w2h = np.ascontiguousarray(np.concatenate([l2_W1, l2_W2], 0).T, dtype=np.float32)
    b2h = np.concatenate([l2_b1, l2_b2])[None, :].astype(np.float32)
    w20 = np.ascontiguousarray(np.asarray(l2_W0).T, dtype=np.float32)
    b20 = np.asarray(l2_b0)[None, :].astype(np.float32)
    bng = np.asarray(bn_gamma)[None, :].astype(np.float32)
    bnb = np.asarray(bn_beta)[None, :].astype(np.float32)
    fpw = np.ascontiguousarray(np.asarray(fp_W).T, dtype=np.float32)
    fpb = np.asarray(fp_b)[None, :].astype(np.float32)

    nc = _build(TA, TN, sched_a, sched_n)
    fix_waits(nc)

    in_maps = []
    for c in range(NCORES):
        in_maps.append({
            "xt": xt, "xl": np.ascontiguousarray(xt[:, c * R:(c + 1) * R]),
            "a_col": a_cols[c], "a_lrow": a_lrows[c], "a_val": a_vals[c],
            "n_col": n_cols[c], "n_lrow": n_lrows[c], "n_val": n_vals[c],
            "iota": iota, "ident": ident, "ones1": ones1, "onesb": onesb,
            "w1h": w1h, "w10": w10, "b1h": b1h, "b10": b10,
            "w2h": w2h, "w20": w20, "b2h": b2h, "b20": b20,
            "bng": bng, "bnb": bnb, "fpw": fpw, "fpb": fpb,
        })
    res = run_bass_kernel_spmd(nc, in_maps, list(range(NCORES)), trace=TRACE)
    LAST_RESULT["res"] = res
    out = np.concatenate([res.results[c]["out"] for c in range(NCORES)], axis=0)
    return out

